# revision 19
# baseline (speedup 1.0000x reference)
"""Trainium2 Bass kernel for gnn_message_passing (N=1024, H=128, L=3 levels).

Sharding: each of 8 NeuronCores owns N/8=128 rows (i) of the N x N pairwise
computation and all N columns (j); updated node features are all-gathered
between levels (one AllGather carries both x and x^T so no extra on-device
transposes of the gathered tensor are needed).

Edge weights ew = LN(silu(scales @ de_W + de_b)) are level-independent: they
are computed once on device (normalized, bf16) into an internal HBM buffer
and streamed back during each level's message loop.

Math per level (per core, i-rows on partitions):
  m_pre[i,(j,h)] = (x_rows @ Wi)  (+)  broadcast(x_all @ Wj + msg_b)[j,h]
    -> PE matmuls into PSUM (Wi replicated BJ times; ones-column broadcast)
  a = silu(m_pre)                       -> ACT
  per-(i,j) LayerNorm stats over h      -> DVE segmented reduces + ACT square
  t_m = (a - mu) * rstd [* g + be]      -> DVE stride-0 broadcast ops
  msum[i,h] += sum_j t_m * t_e          -> DVE bf16 product + j-axis reduce
Then the update net (PE + small LN), AllGather, and a final projection head.
"""
import sys
sys.path.insert(0, '/opt/trn_rl_repo')

import numpy as np
import ml_dtypes

import concourse.bass as bass
import concourse.bacc as bacc
import concourse.mybir as mybir
from concourse import tile
from concourse.bass_utils import run_bass_kernel_spmd

F32 = mybir.dt.float32
BF16 = mybir.dt.bfloat16
AX = mybir.AxisListType
OP = mybir.AluOpType
AF = mybir.ActivationFunctionType

NCORES = 8
N = 1024
H = 128
L = 3
R = N // NCORES          # 128 rows per core
EPS = 1e-5
BJ = 8                   # j's per main-loop iteration
NIT = N // BJ            # iterations per level
NSPLIT = 4               # per-j normalizes on DVE (rest on ACT)


def _seg(ap, s):
    return ap.rearrange("p (s h) -> p s h", s=s)


def _bcast_j(ap, s, h=H):
    return ap.rearrange("p s -> p s ()").to_broadcast([ap.shape[0], s, h])


def _bcast_h(ap, s):
    # [P, H] -> [P, s, H] (replicate along segment axis)
    return ap.rearrange("p h -> p () h").to_broadcast([ap.shape[0], s, ap.shape[1]])


def _jview(ap, s):
    return ap.rearrange("p (s h) -> p h s", s=s)


def build_nc(spec):
    nc = bacc.Bacc("TRN2", target_bir_lowering=False, debug=False,
                   num_devices=NCORES)

    d_xrows0 = nc.dram_tensor("xrows0", [R, H], F32, kind="ExternalInput")
    d_xrowsT0 = nc.dram_tensor("xrowsT0", [H, R], BF16, kind="ExternalInput")
    d_xallT0 = nc.dram_tensor("xallT0", [H, N], BF16, kind="ExternalInput")
    d_s4T = nc.dram_tensor("s4T", [NIT, 4, R * BJ], BF16, kind="ExternalInput")
    d_deW4 = nc.dram_tensor("deW4", [4, H], BF16, kind="ExternalInput")
    d_degbe = nc.dram_tensor("de_gbe", [2, H], F32, kind="ExternalInput")
    d_wi_rep = nc.dram_tensor("wi_rep", [L, H, BJ * H], BF16, kind="ExternalInput")
    d_wj = nc.dram_tensor("wj", [L, H, H], BF16, kind="ExternalInput")
    d_msgb = nc.dram_tensor("msg_b", [L, 1, H], F32, kind="ExternalInput")
    d_msggbe = nc.dram_tensor("msg_gbe", [L, 2, H], F32, kind="ExternalInput")
    d_updw = nc.dram_tensor("updw", [L, 2 * H, H], BF16, kind="ExternalInput")
    d_updb = nc.dram_tensor("upd_b", [L, 1, H], F32, kind="ExternalInput")
    d_updgbe = nc.dram_tensor("upd_gbe", [L, 2, H], F32, kind="ExternalInput")
    d_fpw = nc.dram_tensor("fpw", [L * H, 2 * H], F32, kind="ExternalInput")
    d_fpb = nc.dram_tensor("fp_b", [1, 2 * H], F32, kind="ExternalInput")
    d_fpgbe = nc.dram_tensor("fp_gbe", [2, 2 * H], F32, kind="ExternalInput")
    d_ident = nc.dram_tensor("ident", [128, 128], F32, kind="ExternalInput")
    d_out = nc.dram_tensor("out", [1, 2 * H], F32, kind="ExternalOutput")

    with tile.TileContext(nc) as tc:
        with (
            tc.tile_pool(name="const", bufs=1) as cpool,
            tc.tile_pool(name="lvl", bufs=1) as lpool,
            tc.tile_pool(name="work", bufs=3) as wpool,
            tc.tile_pool(name="abuf", bufs=18) as apool,
            tc.tile_pool(name="stats", bufs=2) as spool,
            tc.tile_pool(name="psum", bufs=2, space="PSUM") as ppool,
            tc.tile_pool(name="psmall", bufs=1, space="PSUM") as pspool,
            tc.tile_pool(name="dram", bufs=1, space="DRAM") as dpool,
        ):
            # ---------- constants ----------
            ident = cpool.tile([128, 128], F32, tag="ident")
            nc.sync.dma_start(ident[:], d_ident[:])
            ones_row = cpool.tile([1, 128], BF16, tag="ones_row")
            nc.vector.memset(ones_row[:], 1.0)
            ones_col = cpool.tile([128, 1], BF16, tag="ones_col")
            nc.vector.memset(ones_col[:], 1.0)
            eps_col = cpool.tile([128, 1], F32, tag="eps_col")
            nc.vector.memset(eps_col[:], EPS)
            deW4 = cpool.tile([4, H], BF16, tag="deW4")
            nc.sync.dma_start(deW4[:], d_deW4[:])
            xallT = cpool.tile([H, N], BF16, tag="xallT")
            nc.sync.dma_start(xallT[:], d_xallT0[:])
            xrows = cpool.tile([R, H], F32, tag="xrows")
            nc.sync.dma_start(xrows[:], d_xrows0[:])
            xrowsT = cpool.tile([H, R], BF16, tag="xrowsT")
            nc.sync.dma_start(xrowsT[:], d_xrowsT0[:])
            lf_sb = cpool.tile([1, L * H], F32, tag="lf")

            def hvec_bcast(dram_ap, tag):
                """[1, H] dram row -> [128, H] SBUF tile replicated across partitions."""
                row = cpool.tile([1, H], F32, tag=tag + "_row")
                nc.sync.dma_start(row[:], dram_ap)
                ps = pspool.tile([128, 128], F32, tag="ps_sm")
                nc.tensor.matmul(ps[:], ones_row[:], row[:], start=True, stop=True)
                t = cpool.tile([128, H], F32, tag=tag)
                nc.scalar.copy(t[:], ps[:])
                return t

            de_g_b = de_be_b = None
            if not spec["de_gbe_trivial"]:
                de_g_b = hvec_bcast(d_degbe[0:1, :], "de_g")
                de_be_b = hvec_bcast(d_degbe[1:2, :], "de_be")
            msg_g_b, msg_be_b, msgb_b = [None] * L, [None] * L, [None] * L
            upd_g_b, upd_be_b, updb_b = [None] * L, [None] * L, [None] * L
            for lvl in range(L):
                if not spec["msg_gbe_trivial"][lvl]:
                    msg_g_b[lvl] = hvec_bcast(d_msggbe[lvl, 0:1, :], f"msg_g{lvl}")
                    msg_be_b[lvl] = hvec_bcast(d_msggbe[lvl, 1:2, :], f"msg_be{lvl}")
                if not spec["msg_b_trivial"][lvl]:
                    msgb_b[lvl] = hvec_bcast(d_msgb[lvl, 0:1, :], f"msg_b{lvl}")
                if not spec["upd_gbe_trivial"][lvl]:
                    upd_g_b[lvl] = hvec_bcast(d_updgbe[lvl, 0:1, :], f"upd_g{lvl}")
                    upd_be_b[lvl] = hvec_bcast(d_updgbe[lvl, 1:2, :], f"upd_be{lvl}")
                if not spec["upd_b_trivial"][lvl]:
                    updb_b[lvl] = hvec_bcast(d_updb[lvl, 0:1, :], f"upd_b{lvl}")

            te_hbm = dpool.tile([128, NIT * BJ * H], BF16, tag="te_hbm")

            G = 8   # iterations per batched-sqrt super-iteration

            def stats_from_bn(bn, sg, pfx):
                """bn [128, sg*6] (even/odd bn_stats) -> (mu, rstd, -mu*rstd)."""
                bv = bn[:].rearrange("p (s x) -> p s x", x=6)
                m_e, cv_e = bv[:, :, 1], bv[:, :, 2]
                m_o, cv_o = bv[:, :, 4], bv[:, :, 5]
                smu = spool.tile([128, sg], F32, tag=pfx + "smu")
                nc.vector.tensor_tensor(smu[:], m_e, m_o, op=OP.add)
                mu = spool.tile([128, sg], F32, tag=pfx + "mu")
                nc.vector.tensor_scalar_mul(mu[:], smu[:], 0.5)
                dd = spool.tile([128, sg], F32, tag=pfx + "dd")
                nc.vector.tensor_tensor(dd[:], m_e, m_o, op=OP.subtract)
                dd2 = spool.tile([128, sg], F32, tag=pfx + "dd2")
                nc.vector.tensor_tensor(dd2[:], dd[:], dd[:], op=OP.mult)
                cv = spool.tile([128, sg], F32, tag=pfx + "cv")
                nc.vector.tensor_tensor(cv[:], cv_e, cv_o, op=OP.add)
                varr = spool.tile([128, sg], F32, tag=pfx + "varr")
                nc.vector.scalar_tensor_tensor(
                    varr[:], dd2[:], float(H / 4), cv[:], op0=OP.mult, op1=OP.add)
                srt = spool.tile([128, sg], F32, tag=pfx + "srt")
                nc.scalar.activation(srt[:], varr[:], AF.Sqrt,
                                     bias=eps_col[:], scale=1.0 / H)
                r = spool.tile([128, sg], F32, tag=pfx + "r")
                nc.vector.reciprocal(r[:], srt[:])
                nmur = spool.tile([128, sg], F32, tag=pfx + "nmur")
                nc.vector.scalar_tensor_tensor(
                    nmur[:], mu[:], -1.0, r[:], op0=OP.mult, op1=OP.mult)
                return mu, r, nmur

            # ---------- stage B: edge-weight precompute ----------
            for g in range(NIT // G):
                a_list = []
                bn = spool.tile([128, G * BJ * 6], F32, tag="bn")
                for u in range(G):
                    t = g * G + u
                    s4c = wpool.tile([4, R * BJ], BF16, tag="s4c")
                    nc.sync.dma_start(s4c[:], d_s4T[t])
                    ps_e = ppool.tile([128, BJ * H], F32, tag="ps_big")
                    s4v = s4c[:].rearrange("k (i j) -> k i j", j=BJ)
                    for jl in range(BJ):
                        nc.tensor.matmul(
                            ps_e[:, jl * H:(jl + 1) * H], s4v[:, :, jl], deW4[:],
                            start=True, stop=True)
                    a = apool.tile([128, BJ * H], BF16, tag="ga")
                    nc.scalar.activation(a[:], ps_e[:], AF.Silu)
                    for j in range(BJ):
                        k = u * BJ + j
                        nc.vector.bn_stats(bn[:, k * 6:(k + 1) * 6],
                                           a[:, j * H:(j + 1) * H])
                    a_list.append(a)
                mu, r, nmur = stats_from_bn(bn, G * BJ, "e")
                for u in range(G):
                    t = g * G + u
                    a = a_list[u]
                    te = wpool.tile([128, BJ * H], BF16, tag="bf_te")
                    for j in range(BJ):
                        k = u * BJ + j
                        if j < NSPLIT:
                            nc.vector.tensor_scalar(
                                te[:, j * H:(j + 1) * H], a[:, j * H:(j + 1) * H],
                                mu[:, k:k + 1], r[:, k:k + 1],
                                op0=OP.subtract, op1=OP.mult)
                        else:
                            nc.scalar.activation(
                                te[:, j * H:(j + 1) * H], a[:, j * H:(j + 1) * H],
                                AF.Identity, bias=nmur[:, k:k + 1], scale=r[:, k:k + 1])
                    if not spec["de_gbe_trivial"]:
                        te2 = wpool.tile([128, BJ * H], BF16, tag="bf_te2")
                        nc.vector.tensor_tensor(
                            _seg(te2[:], BJ), _seg(te[:], BJ),
                            _bcast_h(de_g_b[:], BJ), op=OP.mult)
                        te3 = wpool.tile([128, BJ * H], BF16, tag="bf_te3")
                        nc.vector.tensor_tensor(
                            _seg(te3[:], BJ), _seg(te2[:], BJ),
                            _bcast_h(de_be_b[:], BJ), op=OP.add)
                        te = te3
                    nc.sync.dma_start(te_hbm[:, t * BJ * H:(t + 1) * BJ * H], te[:])

            # ---------- stage C: levels ----------
            for lvl in range(L):
                wi_rep = lpool.tile([H, BJ * H], BF16, tag="wi_rep")
                nc.sync.dma_start(wi_rep[:], d_wi_rep[lvl])
                wj = lpool.tile([H, H], BF16, tag="wj")
                nc.sync.dma_start(wj[:], d_wj[lvl])

                # prjb[t, (g, h)] = (x_all @ Wj + msg_b)[t*BJ+g, h]
                prj_dram = dpool.tile([N, H], BF16, tag=f"prj_dram{lvl}")
                for jb in range(N // 128):
                    ps_p_full = pspool.tile([128, 128], F32, tag="ps_sm")
                    ps_p = ps_p_full[:, 0:H]
                    nc.tensor.matmul(ps_p[:], xallT[:, jb * 128:(jb + 1) * 128],
                                     wj[:], start=True, stop=True)
                    prj_sb = wpool.tile([128, H], BF16, tag="prj_sb")
                    if spec["msg_b_trivial"][lvl]:
                        nc.scalar.copy(prj_sb[:], ps_p[:])
                    else:
                        nc.vector.tensor_tensor(
                            prj_sb[:], ps_p[:], msgb_b[lvl][:], op=OP.add)
                    nc.sync.dma_start(
                        prj_dram[jb * 128:(jb + 1) * 128, :], prj_sb[:])
                prjb_rows = prj_dram[:].rearrange("(t g) h -> t (g h)", g=BJ)

                msum = lpool.tile([R, H], F32, tag="msum")
                nc.vector.memset(msum[:], 0.0)

                for g in range(NIT // G):
                    a_list = []
                    bn = spool.tile([128, G * BJ * 6], F32, tag="bn")
                    for u in range(G):
                        t = g * G + u
                        prjb_t = wpool.tile([1, BJ * H], BF16, tag="prjb_t")
                        nc.sync.dma_start(prjb_t[:], prjb_rows[t:t + 1, :])
                        ps_m = ppool.tile([128, BJ * H], F32, tag="ps_big")
                        half = BJ * H // 2
                        for c0 in range(2):
                            nc.tensor.matmul(
                                ps_m[:, c0 * half:(c0 + 1) * half],
                                xrowsT[:],
                                wi_rep[:, c0 * half:(c0 + 1) * half],
                                start=True, stop=False)
                        for c0 in range(2):
                            nc.tensor.matmul(
                                ps_m[:, c0 * half:(c0 + 1) * half],
                                ones_row[:],
                                prjb_t[:, c0 * half:(c0 + 1) * half],
                                start=False, stop=True)
                        a = apool.tile([128, BJ * H], BF16, tag="ga")
                        nc.scalar.activation(a[:], ps_m[:], AF.Silu)
                        for j in range(BJ):
                            k = u * BJ + j
                            nc.vector.bn_stats(bn[:, k * 6:(k + 1) * 6],
                                               a[:, j * H:(j + 1) * H])
                        a_list.append(a)
                    mu, r, nmur = stats_from_bn(bn, G * BJ, "m")
                    for u in range(G):
                        t = g * G + u
                        a = a_list[u]
                        te = wpool.tile([128, BJ * H], BF16, tag="bf_te")
                        nc.sync.dma_start(
                            te[:], te_hbm[:, t * BJ * H:(t + 1) * BJ * H])
                        tm = wpool.tile([128, BJ * H], BF16, tag="bf_tm")
                        for j in range(BJ):
                            k = u * BJ + j
                            if j < NSPLIT:
                                nc.vector.tensor_scalar(
                                    tm[:, j * H:(j + 1) * H], a[:, j * H:(j + 1) * H],
                                    mu[:, k:k + 1], r[:, k:k + 1],
                                    op0=OP.subtract, op1=OP.mult)
                            else:
                                nc.scalar.activation(
                                    tm[:, j * H:(j + 1) * H], a[:, j * H:(j + 1) * H],
                                    AF.Identity, bias=nmur[:, k:k + 1], scale=r[:, k:k + 1])
                        if not spec["msg_gbe_trivial"][lvl]:
                            tm2 = wpool.tile([128, BJ * H], BF16, tag="bf_tm2")
                            nc.vector.tensor_tensor(
                                _seg(tm2[:], BJ), _seg(tm[:], BJ),
                                _bcast_h(msg_g_b[lvl][:], BJ), op=OP.mult)
                            tm3 = wpool.tile([128, BJ * H], BF16, tag="bf_tm3")
                            nc.vector.tensor_tensor(
                                _seg(tm3[:], BJ), _seg(tm2[:], BJ),
                                _bcast_h(msg_be_b[lvl][:], BJ), op=OP.add)
                            tm = tm3
                        prod = wpool.tile([128, BJ * H], BF16, tag="bf_prod")
                        nc.vector.tensor_tensor(prod[:], tm[:], te[:], op=OP.mult)
                        h1 = wpool.tile([128, BJ * H // 2], BF16, tag="tree1")
                        nc.vector.tensor_tensor(
                            h1[:], prod[:, 0:BJ * H // 2],
                            prod[:, BJ * H // 2:BJ * H], op=OP.add)
                        h2 = wpool.tile([128, BJ * H // 4], BF16, tag="tree2")
                        nc.vector.tensor_tensor(
                            h2[:], h1[:, 0:BJ * H // 4],
                            h1[:, BJ * H // 4:BJ * H // 2], op=OP.add)
                        h3 = wpool.tile([128, H], F32, tag="tree3")
                        nc.vector.tensor_tensor(
                            h3[:], h2[:, 0:H], h2[:, H:2 * H], op=OP.add)
                        nc.vector.tensor_tensor(msum[:], msum[:], h3[:], op=OP.add)

                # ---- update net ----
                ps_t = pspool.tile([128, 128], F32, tag="ps_sm")
                nc.tensor.transpose(ps_t[:], msum[:], ident[:])
                msumT = lpool.tile([H, R], BF16, tag="msumT")
                nc.scalar.copy(msumT[:], ps_t[:])
                w1 = lpool.tile([H, H], BF16, tag="updw1")
                nc.sync.dma_start(w1[:], d_updw[lvl, 0:H, :])
                w2 = lpool.tile([H, H], BF16, tag="updw2")
                nc.sync.dma_start(w2[:], d_updw[lvl, H:2 * H, :])
                ps_u_full = pspool.tile([128, 128], F32, tag="ps_sm")
                ps_u = ps_u_full[:, 0:H]
                nc.tensor.matmul(ps_u[:], xrowsT[:], w1[:], start=True, stop=False)
                nc.tensor.matmul(ps_u[:], msumT[:], w2[:], start=False, stop=True)
                ua = lpool.tile([R, H], F32, tag="ua")
                if spec["upd_b_trivial"][lvl]:
                    nc.scalar.activation(ua[:], ps_u[:], AF.Silu)
                else:
                    ub = lpool.tile([R, H], F32, tag="ub")
                    nc.vector.tensor_tensor(ub[:], ps_u[:], updb_b[lvl][:], op=OP.add)
                    nc.scalar.activation(ua[:], ub[:], AF.Silu)
                # LN over h (per-partition scalars)
                us1 = spool.tile([R, 1], F32, tag="us1")
                nc.vector.reduce_sum(us1[:], ua[:], axis=AX.X)
                usq = lpool.tile([R, H], F32, tag="usq")
                nc.scalar.activation(usq[:], ua[:], AF.Square)
                us2 = spool.tile([R, 1], F32, tag="us2")
                nc.vector.reduce_sum(us2[:], usq[:], axis=AX.X)
                umu = spool.tile([R, 1], F32, tag="umu")
                nc.vector.tensor_scalar_mul(umu[:], us1[:], 1.0 / H)
                umusq = spool.tile([R, 1], F32, tag="umusq")
                nc.vector.tensor_tensor(umusq[:], umu[:], umu[:], op=OP.mult)
                uvar = spool.tile([R, 1], F32, tag="uvar")
                nc.vector.scalar_tensor_tensor(
                    uvar[:], us2[:], 1.0 / H, umusq[:], op0=OP.mult, op1=OP.subtract)
                usrt = spool.tile([R, 1], F32, tag="usrt")
                nc.scalar.activation(usrt[:], uvar[:], AF.Sqrt, bias=eps_col[:])
                ur = spool.tile([R, 1], F32, tag="ur")
                nc.vector.reciprocal(ur[:], usrt[:])
                un = lpool.tile([R, H], F32, tag="un")
                nc.vector.tensor_scalar(un[:], ua[:], umu[:], ur[:],
                                        op0=OP.subtract, op1=OP.mult)
                if not spec["upd_gbe_trivial"][lvl]:
                    un2 = lpool.tile([R, H], F32, tag="un2")
                    nc.vector.tensor_tensor(un2[:], un[:], upd_g_b[lvl][:], op=OP.mult)
                    un3 = lpool.tile([R, H], F32, tag="un3")
                    nc.vector.tensor_tensor(un3[:], un2[:], upd_be_b[lvl][:], op=OP.add)
                    un = un3
                xnew = lpool.tile([R, H], F32, tag="xnew")
                nc.vector.tensor_tensor(xnew[:], xrows[:], un[:], op=OP.add)
                nc.vector.tensor_copy(xrows[:], xnew[:])

                ps_xt = pspool.tile([128, 128], F32, tag="ps_sm")
                nc.tensor.transpose(ps_xt[:], xnew[:], ident[:])
                nc.scalar.copy(xrowsT[:], ps_xt[:])
                xnew_bf = lpool.tile([R, H], BF16, tag="xnew_bf")
                nc.scalar.copy(xnew_bf[:], ps_u[:] if False else xnew[:])

                # ---- AllGather [xnew; xnewT] ----
                ag_in = dpool.tile([2 * R, H], BF16, tag=f"ag_in{lvl}")
                ag_out = dpool.tile([2 * N, H], BF16, tag=f"ag_out{lvl}")
                nc.sync.dma_start(ag_in[0:R, :], xnew_bf[:])
                nc.sync.dma_start(ag_in[R:2 * R, :], xrowsT[:])
                nc.gpsimd.collective_compute(
                    "AllGather", OP.bypass,
                    replica_groups=[list(range(NCORES))],
                    ins=[ag_in.opt()],
                    outs=[ag_out.opt()],
                )
                ps_lf_full = pspool.tile([1, 256], F32, tag="ps_vec")
                ps_lf = ps_lf_full[:, 0:H]
                for c in range(NCORES):
                    nc.sync.dma_start(
                        xallT[:, c * R:(c + 1) * R],
                        ag_out[(2 * c + 1) * R:(2 * c + 2) * R, :])
                    xg = wpool.tile([R, H], BF16, tag="xg")
                    nc.sync.dma_start(xg[:], ag_out[2 * c * R:(2 * c + 1) * R, :])
                    nc.tensor.matmul(ps_lf[:], ones_col[:], xg[:],
                                     start=(c == 0), stop=(c == NCORES - 1))
                nc.scalar.mul(lf_sb[:, lvl * H:(lvl + 1) * H], ps_lf[:], 1.0 / N)

            # ---------- stage D: final projection head ----------
            lf_dram = dpool.tile([1, L * H], F32, tag="lf_dram")
            nc.sync.dma_start(lf_dram[:], lf_sb[:])
            cmbT = cpool.tile([128, L], F32, tag="cmbT")
            nc.sync.dma_start(
                cmbT[:], lf_dram[0, :].rearrange("(l k) -> k l", k=128))
            fpw_sb = cpool.tile([128, L * 2 * H], F32, tag="fpw_sb")
            for l in range(L):
                nc.sync.dma_start(
                    fpw_sb[:, l * 2 * H:(l + 1) * 2 * H],
                    d_fpw[l * 128:(l + 1) * 128, :])
            ps_o = pspool.tile([1, 256], F32, tag="ps_vec")
            for l in range(L):
                nc.tensor.matmul(
                    ps_o[:], cmbT[:, l:l + 1],
                    fpw_sb[:, l * 2 * H:(l + 1) * 2 * H],
                    start=(l == 0), stop=(l == L - 1))
            fpb_sb = cpool.tile([1, 2 * H], F32, tag="fpb_sb")
            nc.sync.dma_start(fpb_sb[:], d_fpb[:])
            f0 = cpool.tile([1, 2 * H], F32, tag="f0")
            nc.vector.tensor_tensor(f0[:], ps_o[:], fpb_sb[:], op=OP.add)
            # LN over 2H on one partition
            fs1 = spool.tile([1, 1], F32, tag="fs1")
            nc.vector.reduce_sum(fs1[:], f0[:], axis=AX.X)
            fsq = cpool.tile([1, 2 * H], F32, tag="fsq")
            nc.scalar.activation(fsq[:], f0[:], AF.Square)
            fs2 = spool.tile([1, 1], F32, tag="fs2")
            nc.vector.reduce_sum(fs2[:], fsq[:], axis=AX.X)
            fmu = spool.tile([1, 1], F32, tag="fmu")
            nc.vector.tensor_scalar_mul(fmu[:], fs1[:], 1.0 / (2 * H))
            fmusq = spool.tile([1, 1], F32, tag="fmusq")
            nc.vector.tensor_tensor(fmusq[:], fmu[:], fmu[:], op=OP.mult)
            fvar = spool.tile([1, 1], F32, tag="fvar")
            nc.vector.scalar_tensor_tensor(
                fvar[:], fs2[:], 1.0 / (2 * H), fmusq[:],
                op0=OP.mult, op1=OP.subtract)
            fsrt = spool.tile([1, 1], F32, tag="fsrt")
            nc.scalar.activation(fsrt[:], fvar[:], AF.Sqrt, bias=eps_col[0:1, :])
            fr = spool.tile([1, 1], F32, tag="fr")
            nc.vector.reciprocal(fr[:], fsrt[:])
            fn = cpool.tile([1, 2 * H], F32, tag="fn")
            nc.vector.tensor_scalar(fn[:], f0[:], fmu[:], fr[:],
                                    op0=OP.subtract, op1=OP.mult)
            if not spec["fp_gbe_trivial"]:
                fg = cpool.tile([1, 2 * H], F32, tag="fg")
                nc.sync.dma_start(fg[:], d_fpgbe[0:1, :])
                fbe = cpool.tile([1, 2 * H], F32, tag="fbe")
                nc.sync.dma_start(fbe[:], d_fpgbe[1:2, :])
                fn2 = cpool.tile([1, 2 * H], F32, tag="fn2")
                nc.vector.tensor_tensor(fn2[:], fn[:], fg[:], op=OP.mult)
                fn3 = cpool.tile([1, 2 * H], F32, tag="fn3")
                nc.vector.tensor_tensor(fn3[:], fn2[:], fbe[:], op=OP.add)
                fn = fn3
            nc.sync.dma_start(d_out[:], fn[:])

    nc.finalize()
    return nc


# ----------------------------------------------------------------------------
# Host side
# ----------------------------------------------------------------------------

_CACHE = {}


def _prep(atomic_numbers, positions, emb, de_W, de_b, de_g, de_be,
          msg_W, msg_b, msg_g, msg_be, upd_W, upd_b, upd_g, upd_be,
          fp_W, fp_b, fp_g, fp_be):
    f = np.asarray
    x0 = f(emb, np.float32)[np.asarray(atomic_numbers).astype(np.int64)]  # [N,H]
    pos = f(positions, np.float32)
    diff = pos[:, None, :] - pos[None, :, :]
    sq = np.sum(diff * diff, axis=-1)
    d = np.sqrt(np.maximum(sq, 0.0), dtype=np.float32)
    np.fill_diagonal(d, 0.0)
    s1 = np.exp(-d, dtype=np.float32)
    s2 = np.exp(-d / 2, dtype=np.float32)
    s3 = np.exp(-d / 4, dtype=np.float32)

    spec = {
        "de_gbe_trivial": bool(np.all(f(de_g) == 1) and np.all(f(de_be) == 0)),
        "msg_b_trivial": [bool(np.all(f(msg_b)[l] == 0)) for l in range(L)],
        "msg_gbe_trivial": [bool(np.all(f(msg_g)[l] == 1) and np.all(f(msg_be)[l] == 0))
                            for l in range(L)],
        "upd_b_trivial": [bool(np.all(f(upd_b)[l] == 0)) for l in range(L)],
        "upd_gbe_trivial": [bool(np.all(f(upd_g)[l] == 1) and np.all(f(upd_be)[l] == 0))
                            for l in range(L)],
        "fp_gbe_trivial": bool(np.all(f(fp_g) == 1) and np.all(f(fp_be) == 0)),
    }

    BF = ml_dtypes.bfloat16
    msg_W = f(msg_W, np.float32)
    wi_rep = np.stack([np.tile(msg_W[l, :H, :], (1, BJ)) for l in range(L)]).astype(BF)
    wj = np.ascontiguousarray(msg_W[:, H:, :]).astype(BF)
    deW4 = np.concatenate([f(de_W, np.float32), f(de_b, np.float32)[None, :]], 0).astype(BF)

    shared = {
        "xallT0": np.ascontiguousarray(x0.T).astype(BF),
        "deW4": np.ascontiguousarray(deW4),
        "de_gbe": np.stack([f(de_g, np.float32), f(de_be, np.float32)]),
        "wi_rep": np.ascontiguousarray(wi_rep),
        "wj": wj,
        "msg_b": np.ascontiguousarray(f(msg_b, np.float32)[:, None, :]),
        "msg_gbe": np.ascontiguousarray(
            np.stack([f(msg_g, np.float32), f(msg_be, np.float32)], axis=1)),
        "updw": np.ascontiguousarray(f(upd_W, np.float32)).astype(BF),
        "upd_b": np.ascontiguousarray(f(upd_b, np.float32)[:, None, :]),
        "upd_gbe": np.ascontiguousarray(
            np.stack([f(upd_g, np.float32), f(upd_be, np.float32)], axis=1)),
        "fpw": np.ascontiguousarray(f(fp_W, np.float32)),
        "fp_b": np.ascontiguousarray(f(fp_b, np.float32)[None, :]),
        "fp_gbe": np.stack([f(fp_g, np.float32), f(fp_be, np.float32)]),
        "ident": np.eye(128, dtype=np.float32),
    }

    in_maps = []
    ones = np.ones((R, N), np.float32)
    for c in range(NCORES):
        rows = slice(c * R, (c + 1) * R)
        # s4T host layout: [NIT, 4, R, BJ] -> [NIT, 4, R*BJ]
        s4 = np.stack([s1[rows], s2[rows], s3[rows], ones])      # [4, R, N]
        s4 = s4.reshape(4, R, NIT, BJ).transpose(2, 0, 1, 3)      # [NIT,4,R,BJ]
        m = dict(shared)
        m["xrows0"] = np.ascontiguousarray(x0[rows])
        m["xrowsT0"] = np.ascontiguousarray(x0[rows].T).astype(BF)
        m["s4T"] = np.ascontiguousarray(s4.reshape(NIT, 4, R * BJ)).astype(BF)
        in_maps.append(m)
    return spec, in_maps


def kernel(**inputs) -> np.ndarray:
    spec, in_maps = _prep(**inputs)
    key = tuple(spec["msg_b_trivial"]) + tuple(spec["msg_gbe_trivial"]) + \
        tuple(spec["upd_b_trivial"]) + tuple(spec["upd_gbe_trivial"]) + \
        (spec["de_gbe_trivial"], spec["fp_gbe_trivial"])
    if key not in _CACHE:
        _CACHE[key] = build_nc(spec)
    nc = _CACHE[key]
    res = run_bass_kernel_spmd(nc, in_maps, core_ids=list(range(NCORES)))
    return res.results[0]["out"].reshape(2 * H).astype(np.float32)


def run_traced(**inputs):
    """Like kernel() but with NTFF tracing; returns (out, BassKernelResults)."""
    import antenv
    extra = '/root/axon_shim/antenv_extra'
    if extra not in antenv.__path__:
        antenv.__path__.append(extra)
    from antenv.axon_hooks import set_axon_ntff_profile_hook, get_axon_ntff_profile_hook
    if get_axon_ntff_profile_hook() is None:
        from trn_agent_boot.trn_boot import _ntff_profile_via_ctypes
        set_axon_ntff_profile_hook(
            _ntff_profile_via_ctypes('/opt/axon/libaxon_pjrt.so'))
    spec, in_maps = _prep(**inputs)
    nc = build_nc(spec)
    res = run_bass_kernel_spmd(nc, in_maps, core_ids=list(range(NCORES)),
                               trace=True)
    return res.results[0]["out"].reshape(2 * H).astype(np.float32), res


# revision 20
# speedup vs baseline: 1.1940x; 1.1940x over previous
"""Trainium2 Bass kernel for gnn_message_passing (N=1024, H=128, L=3 levels).

Sharding: each of 8 NeuronCores owns N/8=128 rows (i) of the N x N pairwise
computation and all N columns (j); updated node features are all-gathered
between levels (one AllGather carries both x and x^T so no extra on-device
transposes of the gathered tensor are needed).

Edge weights ew = LN(silu(scales @ de_W + de_b)) are level-independent: they
are computed once on device (normalized, bf16) into an internal HBM buffer
and streamed back during each level's message loop.

Math per level (per core, i-rows on partitions):
  m_pre[i,(j,h)] = (x_rows @ Wi)  (+)  broadcast(x_all @ Wj + msg_b)[j,h]
    -> PE matmuls into PSUM (Wi replicated BJ times; ones-column broadcast)
  a = silu(m_pre)                       -> ACT
  per-(i,j) LayerNorm stats over h      -> DVE segmented reduces + ACT square
  t_m = (a - mu) * rstd [* g + be]      -> DVE stride-0 broadcast ops
  msum[i,h] += sum_j t_m * t_e          -> DVE bf16 product + j-axis reduce
Then the update net (PE + small LN), AllGather, and a final projection head.
"""
import sys
sys.path.insert(0, '/opt/trn_rl_repo')

import numpy as np
import ml_dtypes

import concourse.bass as bass
import concourse.bacc as bacc
import concourse.mybir as mybir
from concourse import tile
from concourse.bass_utils import run_bass_kernel_spmd

F32 = mybir.dt.float32
BF16 = mybir.dt.bfloat16
AX = mybir.AxisListType
OP = mybir.AluOpType
AF = mybir.ActivationFunctionType

NCORES = 8
N = 1024
H = 128
L = 3
R = N // NCORES          # 128 rows per core
EPS = 1e-5
BJ = 8                   # j's per main-loop iteration
NIT = N // BJ            # iterations per level
NSPLIT = 4               # per-j normalizes on DVE (rest on ACT)


def _seg(ap, s):
    return ap.rearrange("p (s h) -> p s h", s=s)


def _bcast_j(ap, s, h=H):
    return ap.rearrange("p s -> p s ()").to_broadcast([ap.shape[0], s, h])


def _bcast_h(ap, s):
    # [P, H] -> [P, s, H] (replicate along segment axis)
    return ap.rearrange("p h -> p () h").to_broadcast([ap.shape[0], s, ap.shape[1]])


def _jview(ap, s):
    return ap.rearrange("p (s h) -> p h s", s=s)


def build_nc(spec):
    nc = bacc.Bacc("TRN2", target_bir_lowering=False, debug=False,
                   num_devices=NCORES)

    d_xrows0 = nc.dram_tensor("xrows0", [R, H], F32, kind="ExternalInput")
    d_xrowsT0 = nc.dram_tensor("xrowsT0", [H, R], BF16, kind="ExternalInput")
    d_xallT0 = nc.dram_tensor("xallT0", [H, N], BF16, kind="ExternalInput")
    d_s4T = nc.dram_tensor("s4T", [NIT, 4, R * BJ], BF16, kind="ExternalInput")
    d_deW4 = nc.dram_tensor("deW4", [4, H], BF16, kind="ExternalInput")
    d_degbe = nc.dram_tensor("de_gbe", [2, H], F32, kind="ExternalInput")
    d_wi_rep = nc.dram_tensor("wi_rep", [L, H, BJ * H], BF16, kind="ExternalInput")
    d_wj = nc.dram_tensor("wj", [L, H, H], BF16, kind="ExternalInput")
    d_msgb = nc.dram_tensor("msg_b", [L, 1, H], F32, kind="ExternalInput")
    d_msggbe = nc.dram_tensor("msg_gbe", [L, 2, H], F32, kind="ExternalInput")
    d_updw = nc.dram_tensor("updw", [L, 2 * H, H], BF16, kind="ExternalInput")
    d_updb = nc.dram_tensor("upd_b", [L, 1, H], F32, kind="ExternalInput")
    d_updgbe = nc.dram_tensor("upd_gbe", [L, 2, H], F32, kind="ExternalInput")
    d_fpw = nc.dram_tensor("fpw", [L * H, 2 * H], F32, kind="ExternalInput")
    d_fpb = nc.dram_tensor("fp_b", [1, 2 * H], F32, kind="ExternalInput")
    d_fpgbe = nc.dram_tensor("fp_gbe", [2, 2 * H], F32, kind="ExternalInput")
    d_ident = nc.dram_tensor("ident", [128, 128], F32, kind="ExternalInput")
    d_out = nc.dram_tensor("out", [1, 2 * H], F32, kind="ExternalOutput")

    with tile.TileContext(nc) as tc:
        with (
            tc.tile_pool(name="const", bufs=1) as cpool,
            tc.tile_pool(name="lvl", bufs=1) as lpool,
            tc.tile_pool(name="work", bufs=2) as wpool,
            tc.tile_pool(name="abuf", bufs=18) as apool,
            tc.tile_pool(name="stats", bufs=2) as spool,
            tc.tile_pool(name="psum", bufs=2, space="PSUM") as ppool,
            tc.tile_pool(name="psmall", bufs=1, space="PSUM") as pspool,
            tc.tile_pool(name="dram", bufs=1, space="DRAM") as dpool,
        ):
            # ---------- constants ----------
            ident = cpool.tile([128, 128], F32, tag="ident")
            nc.sync.dma_start(ident[:], d_ident[:])
            ones_row = cpool.tile([1, 128], BF16, tag="ones_row")
            nc.vector.memset(ones_row[:], 1.0)
            ones_col = cpool.tile([128, 1], BF16, tag="ones_col")
            nc.vector.memset(ones_col[:], 1.0)
            eps_col = cpool.tile([128, 1], F32, tag="eps_col")
            nc.vector.memset(eps_col[:], EPS)
            deW4 = cpool.tile([4, H], BF16, tag="deW4")
            nc.sync.dma_start(deW4[:], d_deW4[:])
            xallT = cpool.tile([H, N], BF16, tag="xallT")
            nc.sync.dma_start(xallT[:], d_xallT0[:])
            xrows = cpool.tile([R, H], F32, tag="xrows")
            nc.sync.dma_start(xrows[:], d_xrows0[:])
            xrowsT = cpool.tile([H, R], BF16, tag="xrowsT")
            nc.sync.dma_start(xrowsT[:], d_xrowsT0[:])
            lf_sb = cpool.tile([1, L * H], F32, tag="lf")

            def hvec_bcast(dram_ap, tag):
                """[1, H] dram row -> [128, H] SBUF tile replicated across partitions."""
                row = cpool.tile([1, H], F32, tag=tag + "_row")
                nc.sync.dma_start(row[:], dram_ap)
                ps = pspool.tile([128, 128], F32, tag="ps_sm")
                nc.tensor.matmul(ps[:], ones_row[:], row[:], start=True, stop=True)
                t = cpool.tile([128, H], F32, tag=tag)
                nc.scalar.copy(t[:], ps[:])
                return t

            de_g_b = de_be_b = None
            if not spec["de_gbe_trivial"]:
                de_g_b = hvec_bcast(d_degbe[0:1, :], "de_g")
                de_be_b = hvec_bcast(d_degbe[1:2, :], "de_be")
            msg_g_b, msg_be_b, msgb_b = [None] * L, [None] * L, [None] * L
            upd_g_b, upd_be_b, updb_b = [None] * L, [None] * L, [None] * L
            for lvl in range(L):
                if not spec["msg_gbe_trivial"][lvl]:
                    msg_g_b[lvl] = hvec_bcast(d_msggbe[lvl, 0:1, :], f"msg_g{lvl}")
                    msg_be_b[lvl] = hvec_bcast(d_msggbe[lvl, 1:2, :], f"msg_be{lvl}")
                if not spec["msg_b_trivial"][lvl]:
                    msgb_b[lvl] = hvec_bcast(d_msgb[lvl, 0:1, :], f"msg_b{lvl}")
                if not spec["upd_gbe_trivial"][lvl]:
                    upd_g_b[lvl] = hvec_bcast(d_updgbe[lvl, 0:1, :], f"upd_g{lvl}")
                    upd_be_b[lvl] = hvec_bcast(d_updgbe[lvl, 1:2, :], f"upd_be{lvl}")
                if not spec["upd_b_trivial"][lvl]:
                    updb_b[lvl] = hvec_bcast(d_updb[lvl, 0:1, :], f"upd_b{lvl}")

            te_hbm = dpool.tile([128, NIT * BJ * H], BF16, tag="te_hbm")

            G = 8   # iterations per batched-sqrt super-iteration

            def stats_from_bn(bn, sg, pfx):
                """bn [128, sg*6] (even/odd bn_stats) -> (mu, rstd, -mu*rstd)."""
                bv = bn[:].rearrange("p (s x) -> p s x", x=6)
                m_e, cv_e = bv[:, :, 1], bv[:, :, 2]
                m_o, cv_o = bv[:, :, 4], bv[:, :, 5]
                smu = spool.tile([128, sg], F32, tag=pfx + "smu")
                nc.vector.tensor_tensor(smu[:], m_e, m_o, op=OP.add)
                mu = spool.tile([128, sg], F32, tag=pfx + "mu")
                nc.vector.tensor_scalar_mul(mu[:], smu[:], 0.5)
                dd = spool.tile([128, sg], F32, tag=pfx + "dd")
                nc.vector.tensor_tensor(dd[:], m_e, m_o, op=OP.subtract)
                dd2 = spool.tile([128, sg], F32, tag=pfx + "dd2")
                nc.vector.tensor_tensor(dd2[:], dd[:], dd[:], op=OP.mult)
                cv = spool.tile([128, sg], F32, tag=pfx + "cv")
                nc.vector.tensor_tensor(cv[:], cv_e, cv_o, op=OP.add)
                varr = spool.tile([128, sg], F32, tag=pfx + "varr")
                nc.vector.scalar_tensor_tensor(
                    varr[:], dd2[:], float(H / 4), cv[:], op0=OP.mult, op1=OP.add)
                srt = spool.tile([128, sg], F32, tag=pfx + "srt")
                nc.scalar.activation(srt[:], varr[:], AF.Sqrt,
                                     bias=eps_col[:], scale=1.0 / H)
                r = spool.tile([128, sg], F32, tag=pfx + "r")
                nc.vector.reciprocal(r[:], srt[:])
                nmur = spool.tile([128, sg], F32, tag=pfx + "nmur")
                nc.vector.scalar_tensor_tensor(
                    nmur[:], mu[:], -1.0, r[:], op0=OP.mult, op1=OP.mult)
                return mu, r, nmur

            # ---------- stage B: edge-weight precompute ----------
            for g in range(NIT // G):
                a_list = []
                bn = spool.tile([128, G * BJ * 6], F32, tag="bn")
                for u in range(G):
                    t = g * G + u
                    s4c = wpool.tile([4, R * BJ], BF16, tag="s4c")
                    nc.sync.dma_start(s4c[:], d_s4T[t])
                    ps_e = ppool.tile([128, BJ * H], F32, tag="ps_big")
                    s4v = s4c[:].rearrange("k (i j) -> k i j", j=BJ)
                    for jl in range(BJ):
                        nc.tensor.matmul(
                            ps_e[:, jl * H:(jl + 1) * H], s4v[:, :, jl], deW4[:],
                            start=True, stop=True)
                    a = apool.tile([128, BJ * H], BF16, tag="ga")
                    nc.scalar.activation(a[:], ps_e[:], AF.Silu)
                    for j in range(BJ):
                        k = u * BJ + j
                        nc.vector.bn_stats(bn[:, k * 6:(k + 1) * 6],
                                           a[:, j * H:(j + 1) * H])
                    a_list.append(a)
                mu, r, nmur = stats_from_bn(bn, G * BJ, "e")
                for u in range(G):
                    t = g * G + u
                    a = a_list[u]
                    te = wpool.tile([128, BJ * H], BF16, tag="bf_te")
                    for j in range(BJ):
                        k = u * BJ + j
                        if j < NSPLIT:
                            nc.vector.tensor_scalar(
                                te[:, j * H:(j + 1) * H], a[:, j * H:(j + 1) * H],
                                mu[:, k:k + 1], r[:, k:k + 1],
                                op0=OP.subtract, op1=OP.mult)
                        else:
                            nc.scalar.activation(
                                te[:, j * H:(j + 1) * H], a[:, j * H:(j + 1) * H],
                                AF.Identity, bias=nmur[:, k:k + 1], scale=r[:, k:k + 1])
                    if not spec["de_gbe_trivial"]:
                        te2 = wpool.tile([128, BJ * H], BF16, tag="bf_te2")
                        nc.vector.tensor_tensor(
                            _seg(te2[:], BJ), _seg(te[:], BJ),
                            _bcast_h(de_g_b[:], BJ), op=OP.mult)
                        te3 = wpool.tile([128, BJ * H], BF16, tag="bf_te3")
                        nc.vector.tensor_tensor(
                            _seg(te3[:], BJ), _seg(te2[:], BJ),
                            _bcast_h(de_be_b[:], BJ), op=OP.add)
                        te = te3
                    nc.sync.dma_start(te_hbm[:, t * BJ * H:(t + 1) * BJ * H], te[:])

            # ---------- stage C: levels ----------
            for lvl in range(L):
                wi_rep = lpool.tile([H, BJ * H], BF16, tag="wi_rep")
                nc.sync.dma_start(wi_rep[:], d_wi_rep[lvl])
                wj = lpool.tile([H, H], BF16, tag="wj")
                nc.sync.dma_start(wj[:], d_wj[lvl])

                # prjb[t, (g, h)] = (x_all @ Wj + msg_b)[t*BJ+g, h]
                prj_dram = dpool.tile([N, H], BF16, tag=f"prj_dram{lvl}")
                for jb in range(N // 128):
                    ps_p_full = pspool.tile([128, 128], F32, tag="ps_sm")
                    ps_p = ps_p_full[:, 0:H]
                    nc.tensor.matmul(ps_p[:], xallT[:, jb * 128:(jb + 1) * 128],
                                     wj[:], start=True, stop=True)
                    prj_sb = wpool.tile([128, H], BF16, tag="prj_sb")
                    if spec["msg_b_trivial"][lvl]:
                        nc.scalar.copy(prj_sb[:], ps_p[:])
                    else:
                        nc.vector.tensor_tensor(
                            prj_sb[:], ps_p[:], msgb_b[lvl][:], op=OP.add)
                    nc.sync.dma_start(
                        prj_dram[jb * 128:(jb + 1) * 128, :], prj_sb[:])
                prjb_rows = prj_dram[:].rearrange("(t g) h -> t (g h)", g=BJ)

                msum = lpool.tile([R, H], F32, tag="msum")
                nc.vector.memset(msum[:], 0.0)

                for g in range(NIT // G):
                    a_list = []
                    bn = spool.tile([128, G * BJ * 6], F32, tag="bn")
                    for u in range(G):
                        t = g * G + u
                        prjb_t = wpool.tile([1, BJ * H], BF16, tag="prjb_t")
                        nc.sync.dma_start(prjb_t[:], prjb_rows[t:t + 1, :])
                        ps_m = ppool.tile([128, BJ * H], F32, tag="ps_big")
                        half = BJ * H // 2
                        for c0 in range(2):
                            nc.tensor.matmul(
                                ps_m[:, c0 * half:(c0 + 1) * half],
                                xrowsT[:],
                                wi_rep[:, c0 * half:(c0 + 1) * half],
                                start=True, stop=False)
                        for c0 in range(2):
                            nc.tensor.matmul(
                                ps_m[:, c0 * half:(c0 + 1) * half],
                                ones_row[:],
                                prjb_t[:, c0 * half:(c0 + 1) * half],
                                start=False, stop=True)
                        a = apool.tile([128, BJ * H], BF16, tag="ga")
                        nc.scalar.activation(a[:], ps_m[:], AF.Silu)
                        for j in range(BJ):
                            k = u * BJ + j
                            nc.vector.bn_stats(bn[:, k * 6:(k + 1) * 6],
                                               a[:, j * H:(j + 1) * H])
                        a_list.append(a)
                    mu, r, nmur = stats_from_bn(bn, G * BJ, "m")
                    for u in range(G):
                        t = g * G + u
                        a = a_list[u]
                        te = wpool.tile([128, BJ * H], BF16, tag="bf_te")
                        nc.sync.dma_start(
                            te[:], te_hbm[:, t * BJ * H:(t + 1) * BJ * H])
                        tm = wpool.tile([128, BJ * H], BF16, tag="bf_tm")
                        for j in range(BJ):
                            k = u * BJ + j
                            if j < NSPLIT:
                                nc.vector.tensor_scalar(
                                    tm[:, j * H:(j + 1) * H], a[:, j * H:(j + 1) * H],
                                    mu[:, k:k + 1], r[:, k:k + 1],
                                    op0=OP.subtract, op1=OP.mult)
                            else:
                                nc.scalar.activation(
                                    tm[:, j * H:(j + 1) * H], a[:, j * H:(j + 1) * H],
                                    AF.Identity, bias=nmur[:, k:k + 1], scale=r[:, k:k + 1])
                        if not spec["msg_gbe_trivial"][lvl]:
                            tm2 = wpool.tile([128, BJ * H], BF16, tag="bf_tm2")
                            nc.vector.tensor_tensor(
                                _seg(tm2[:], BJ), _seg(tm[:], BJ),
                                _bcast_h(msg_g_b[lvl][:], BJ), op=OP.mult)
                            tm3 = wpool.tile([128, BJ * H], BF16, tag="bf_tm3")
                            nc.vector.tensor_tensor(
                                _seg(tm3[:], BJ), _seg(tm2[:], BJ),
                                _bcast_h(msg_be_b[lvl][:], BJ), op=OP.add)
                            tm = tm3
                        prod = wpool.tile([128, BJ * H], BF16, tag="bf_prod")
                        nc.vector.tensor_tensor(prod[:], tm[:], te[:], op=OP.mult)
                        h1 = wpool.tile([128, BJ * H // 2], BF16, tag="tree1")
                        nc.vector.tensor_tensor(
                            h1[:], prod[:, 0:BJ * H // 2],
                            prod[:, BJ * H // 2:BJ * H], op=OP.add)
                        h2 = wpool.tile([128, BJ * H // 4], BF16, tag="tree2")
                        nc.vector.tensor_tensor(
                            h2[:], h1[:, 0:BJ * H // 4],
                            h1[:, BJ * H // 4:BJ * H // 2], op=OP.add)
                        h3 = wpool.tile([128, H], F32, tag="tree3")
                        nc.vector.tensor_tensor(
                            h3[:], h2[:, 0:H], h2[:, H:2 * H], op=OP.add)
                        nc.vector.tensor_tensor(msum[:], msum[:], h3[:], op=OP.add)

                # ---- update net ----
                ps_t = pspool.tile([128, 128], F32, tag="ps_sm")
                nc.tensor.transpose(ps_t[:], msum[:], ident[:])
                msumT = lpool.tile([H, R], BF16, tag="msumT")
                nc.scalar.copy(msumT[:], ps_t[:])
                w1 = lpool.tile([H, H], BF16, tag="updw1")
                nc.sync.dma_start(w1[:], d_updw[lvl, 0:H, :])
                w2 = lpool.tile([H, H], BF16, tag="updw2")
                nc.sync.dma_start(w2[:], d_updw[lvl, H:2 * H, :])
                ps_u_full = pspool.tile([128, 128], F32, tag="ps_sm")
                ps_u = ps_u_full[:, 0:H]
                nc.tensor.matmul(ps_u[:], xrowsT[:], w1[:], start=True, stop=False)
                nc.tensor.matmul(ps_u[:], msumT[:], w2[:], start=False, stop=True)
                ua = lpool.tile([R, H], F32, tag="ua")
                if spec["upd_b_trivial"][lvl]:
                    nc.scalar.activation(ua[:], ps_u[:], AF.Silu)
                else:
                    ub = lpool.tile([R, H], F32, tag="ub")
                    nc.vector.tensor_tensor(ub[:], ps_u[:], updb_b[lvl][:], op=OP.add)
                    nc.scalar.activation(ua[:], ub[:], AF.Silu)
                # LN over h (per-partition scalars)
                us1 = spool.tile([R, 1], F32, tag="us1")
                nc.vector.reduce_sum(us1[:], ua[:], axis=AX.X)
                usq = lpool.tile([R, H], F32, tag="usq")
                nc.scalar.activation(usq[:], ua[:], AF.Square)
                us2 = spool.tile([R, 1], F32, tag="us2")
                nc.vector.reduce_sum(us2[:], usq[:], axis=AX.X)
                umu = spool.tile([R, 1], F32, tag="umu")
                nc.vector.tensor_scalar_mul(umu[:], us1[:], 1.0 / H)
                umusq = spool.tile([R, 1], F32, tag="umusq")
                nc.vector.tensor_tensor(umusq[:], umu[:], umu[:], op=OP.mult)
                uvar = spool.tile([R, 1], F32, tag="uvar")
                nc.vector.scalar_tensor_tensor(
                    uvar[:], us2[:], 1.0 / H, umusq[:], op0=OP.mult, op1=OP.subtract)
                usrt = spool.tile([R, 1], F32, tag="usrt")
                nc.scalar.activation(usrt[:], uvar[:], AF.Sqrt, bias=eps_col[:])
                ur = spool.tile([R, 1], F32, tag="ur")
                nc.vector.reciprocal(ur[:], usrt[:])
                un = lpool.tile([R, H], F32, tag="un")
                nc.vector.tensor_scalar(un[:], ua[:], umu[:], ur[:],
                                        op0=OP.subtract, op1=OP.mult)
                if not spec["upd_gbe_trivial"][lvl]:
                    un2 = lpool.tile([R, H], F32, tag="un2")
                    nc.vector.tensor_tensor(un2[:], un[:], upd_g_b[lvl][:], op=OP.mult)
                    un3 = lpool.tile([R, H], F32, tag="un3")
                    nc.vector.tensor_tensor(un3[:], un2[:], upd_be_b[lvl][:], op=OP.add)
                    un = un3
                xnew = lpool.tile([R, H], F32, tag="xnew")
                nc.vector.tensor_tensor(xnew[:], xrows[:], un[:], op=OP.add)
                nc.vector.tensor_copy(xrows[:], xnew[:])

                ps_xt = pspool.tile([128, 128], F32, tag="ps_sm")
                nc.tensor.transpose(ps_xt[:], xnew[:], ident[:])
                nc.scalar.copy(xrowsT[:], ps_xt[:])
                xnew_bf = lpool.tile([R, H], BF16, tag="xnew_bf")
                nc.scalar.copy(xnew_bf[:], ps_u[:] if False else xnew[:])

                # ---- AllGather [xnew; xnewT] ----
                ag_in = dpool.tile([2 * R, H], BF16, tag=f"ag_in{lvl}")
                ag_out = dpool.tile([2 * N, H], BF16, tag=f"ag_out{lvl}")
                nc.sync.dma_start(ag_in[0:R, :], xnew_bf[:])
                nc.sync.dma_start(ag_in[R:2 * R, :], xrowsT[:])
                nc.gpsimd.collective_compute(
                    "AllGather", OP.bypass,
                    replica_groups=[list(range(NCORES))],
                    ins=[ag_in.opt()],
                    outs=[ag_out.opt()],
                )
                ps_lf_full = pspool.tile([1, 256], F32, tag="ps_vec")
                ps_lf = ps_lf_full[:, 0:H]
                for c in range(NCORES):
                    nc.sync.dma_start(
                        xallT[:, c * R:(c + 1) * R],
                        ag_out[(2 * c + 1) * R:(2 * c + 2) * R, :])
                    xg = wpool.tile([R, H], BF16, tag="xg")
                    nc.sync.dma_start(xg[:], ag_out[2 * c * R:(2 * c + 1) * R, :])
                    nc.tensor.matmul(ps_lf[:], ones_col[:], xg[:],
                                     start=(c == 0), stop=(c == NCORES - 1))
                nc.scalar.mul(lf_sb[:, lvl * H:(lvl + 1) * H], ps_lf[:], 1.0 / N)

            # ---------- stage D: final projection head ----------
            lf_dram = dpool.tile([1, L * H], F32, tag="lf_dram")
            nc.sync.dma_start(lf_dram[:], lf_sb[:])
            cmbT = cpool.tile([128, L], F32, tag="cmbT")
            nc.sync.dma_start(
                cmbT[:], lf_dram[0, :].rearrange("(l k) -> k l", k=128))
            fpw_sb = cpool.tile([128, L * 2 * H], F32, tag="fpw_sb")
            for l in range(L):
                nc.sync.dma_start(
                    fpw_sb[:, l * 2 * H:(l + 1) * 2 * H],
                    d_fpw[l * 128:(l + 1) * 128, :])
            ps_o = pspool.tile([1, 256], F32, tag="ps_vec")
            for l in range(L):
                nc.tensor.matmul(
                    ps_o[:], cmbT[:, l:l + 1],
                    fpw_sb[:, l * 2 * H:(l + 1) * 2 * H],
                    start=(l == 0), stop=(l == L - 1))
            fpb_sb = cpool.tile([1, 2 * H], F32, tag="fpb_sb")
            nc.sync.dma_start(fpb_sb[:], d_fpb[:])
            f0 = cpool.tile([1, 2 * H], F32, tag="f0")
            nc.vector.tensor_tensor(f0[:], ps_o[:], fpb_sb[:], op=OP.add)
            # LN over 2H on one partition
            fs1 = spool.tile([1, 1], F32, tag="fs1")
            nc.vector.reduce_sum(fs1[:], f0[:], axis=AX.X)
            fsq = cpool.tile([1, 2 * H], F32, tag="fsq")
            nc.scalar.activation(fsq[:], f0[:], AF.Square)
            fs2 = spool.tile([1, 1], F32, tag="fs2")
            nc.vector.reduce_sum(fs2[:], fsq[:], axis=AX.X)
            fmu = spool.tile([1, 1], F32, tag="fmu")
            nc.vector.tensor_scalar_mul(fmu[:], fs1[:], 1.0 / (2 * H))
            fmusq = spool.tile([1, 1], F32, tag="fmusq")
            nc.vector.tensor_tensor(fmusq[:], fmu[:], fmu[:], op=OP.mult)
            fvar = spool.tile([1, 1], F32, tag="fvar")
            nc.vector.scalar_tensor_tensor(
                fvar[:], fs2[:], 1.0 / (2 * H), fmusq[:],
                op0=OP.mult, op1=OP.subtract)
            fsrt = spool.tile([1, 1], F32, tag="fsrt")
            nc.scalar.activation(fsrt[:], fvar[:], AF.Sqrt, bias=eps_col[0:1, :])
            fr = spool.tile([1, 1], F32, tag="fr")
            nc.vector.reciprocal(fr[:], fsrt[:])
            fn = cpool.tile([1, 2 * H], F32, tag="fn")
            nc.vector.tensor_scalar(fn[:], f0[:], fmu[:], fr[:],
                                    op0=OP.subtract, op1=OP.mult)
            if not spec["fp_gbe_trivial"]:
                fg = cpool.tile([1, 2 * H], F32, tag="fg")
                nc.sync.dma_start(fg[:], d_fpgbe[0:1, :])
                fbe = cpool.tile([1, 2 * H], F32, tag="fbe")
                nc.sync.dma_start(fbe[:], d_fpgbe[1:2, :])
                fn2 = cpool.tile([1, 2 * H], F32, tag="fn2")
                nc.vector.tensor_tensor(fn2[:], fn[:], fg[:], op=OP.mult)
                fn3 = cpool.tile([1, 2 * H], F32, tag="fn3")
                nc.vector.tensor_tensor(fn3[:], fn2[:], fbe[:], op=OP.add)
                fn = fn3
            nc.sync.dma_start(d_out[:], fn[:])

    nc.finalize()
    return nc


# ----------------------------------------------------------------------------
# Host side
# ----------------------------------------------------------------------------

_CACHE = {}


def _prep(atomic_numbers, positions, emb, de_W, de_b, de_g, de_be,
          msg_W, msg_b, msg_g, msg_be, upd_W, upd_b, upd_g, upd_be,
          fp_W, fp_b, fp_g, fp_be):
    f = np.asarray
    x0 = f(emb, np.float32)[np.asarray(atomic_numbers).astype(np.int64)]  # [N,H]
    pos = f(positions, np.float32)
    diff = pos[:, None, :] - pos[None, :, :]
    sq = np.sum(diff * diff, axis=-1)
    d = np.sqrt(np.maximum(sq, 0.0), dtype=np.float32)
    np.fill_diagonal(d, 0.0)
    s1 = np.exp(-d, dtype=np.float32)
    s2 = np.exp(-d / 2, dtype=np.float32)
    s3 = np.exp(-d / 4, dtype=np.float32)

    spec = {
        "de_gbe_trivial": bool(np.all(f(de_g) == 1) and np.all(f(de_be) == 0)),
        "msg_b_trivial": [bool(np.all(f(msg_b)[l] == 0)) for l in range(L)],
        "msg_gbe_trivial": [bool(np.all(f(msg_g)[l] == 1) and np.all(f(msg_be)[l] == 0))
                            for l in range(L)],
        "upd_b_trivial": [bool(np.all(f(upd_b)[l] == 0)) for l in range(L)],
        "upd_gbe_trivial": [bool(np.all(f(upd_g)[l] == 1) and np.all(f(upd_be)[l] == 0))
                            for l in range(L)],
        "fp_gbe_trivial": bool(np.all(f(fp_g) == 1) and np.all(f(fp_be) == 0)),
    }

    BF = ml_dtypes.bfloat16
    msg_W = f(msg_W, np.float32)
    wi_rep = np.stack([np.tile(msg_W[l, :H, :], (1, BJ)) for l in range(L)]).astype(BF)
    wj = np.ascontiguousarray(msg_W[:, H:, :]).astype(BF)
    deW4 = np.concatenate([f(de_W, np.float32), f(de_b, np.float32)[None, :]], 0).astype(BF)

    shared = {
        "xallT0": np.ascontiguousarray(x0.T).astype(BF),
        "deW4": np.ascontiguousarray(deW4),
        "de_gbe": np.stack([f(de_g, np.float32), f(de_be, np.float32)]),
        "wi_rep": np.ascontiguousarray(wi_rep),
        "wj": wj,
        "msg_b": np.ascontiguousarray(f(msg_b, np.float32)[:, None, :]),
        "msg_gbe": np.ascontiguousarray(
            np.stack([f(msg_g, np.float32), f(msg_be, np.float32)], axis=1)),
        "updw": np.ascontiguousarray(f(upd_W, np.float32)).astype(BF),
        "upd_b": np.ascontiguousarray(f(upd_b, np.float32)[:, None, :]),
        "upd_gbe": np.ascontiguousarray(
            np.stack([f(upd_g, np.float32), f(upd_be, np.float32)], axis=1)),
        "fpw": np.ascontiguousarray(f(fp_W, np.float32)),
        "fp_b": np.ascontiguousarray(f(fp_b, np.float32)[None, :]),
        "fp_gbe": np.stack([f(fp_g, np.float32), f(fp_be, np.float32)]),
        "ident": np.eye(128, dtype=np.float32),
    }

    in_maps = []
    ones = np.ones((R, N), np.float32)
    for c in range(NCORES):
        rows = slice(c * R, (c + 1) * R)
        # s4T host layout: [NIT, 4, R, BJ] -> [NIT, 4, R*BJ]
        s4 = np.stack([s1[rows], s2[rows], s3[rows], ones])      # [4, R, N]
        s4 = s4.reshape(4, R, NIT, BJ).transpose(2, 0, 1, 3)      # [NIT,4,R,BJ]
        m = dict(shared)
        m["xrows0"] = np.ascontiguousarray(x0[rows])
        m["xrowsT0"] = np.ascontiguousarray(x0[rows].T).astype(BF)
        m["s4T"] = np.ascontiguousarray(s4.reshape(NIT, 4, R * BJ)).astype(BF)
        in_maps.append(m)
    return spec, in_maps


def kernel(**inputs) -> np.ndarray:
    spec, in_maps = _prep(**inputs)
    key = tuple(spec["msg_b_trivial"]) + tuple(spec["msg_gbe_trivial"]) + \
        tuple(spec["upd_b_trivial"]) + tuple(spec["upd_gbe_trivial"]) + \
        (spec["de_gbe_trivial"], spec["fp_gbe_trivial"])
    if key not in _CACHE:
        _CACHE[key] = build_nc(spec)
    nc = _CACHE[key]
    res = run_bass_kernel_spmd(nc, in_maps, core_ids=list(range(NCORES)))
    return res.results[0]["out"].reshape(2 * H).astype(np.float32)


def run_traced(**inputs):
    """Like kernel() but with NTFF tracing; returns (out, BassKernelResults)."""
    import antenv
    extra = '/root/axon_shim/antenv_extra'
    if extra not in antenv.__path__:
        antenv.__path__.append(extra)
    from antenv.axon_hooks import set_axon_ntff_profile_hook, get_axon_ntff_profile_hook
    if get_axon_ntff_profile_hook() is None:
        from trn_agent_boot.trn_boot import _ntff_profile_via_ctypes
        set_axon_ntff_profile_hook(
            _ntff_profile_via_ctypes('/opt/axon/libaxon_pjrt.so'))
    spec, in_maps = _prep(**inputs)
    nc = build_nc(spec)
    res = run_bass_kernel_spmd(nc, in_maps, core_ids=list(range(NCORES)),
                               trace=True)
    return res.results[0]["out"].reshape(2 * H).astype(np.float32), res
